# revision 12
# baseline (speedup 1.0000x reference)
"""Trainium2 Bass kernel for a dense transformer block (B=8, N=1024, C=1024,
H=16, D=64, HID=4096) with padding-masked attention.

Sharding: data-parallel over batch - one batch element per NeuronCore.
Transposed layout ([C partitions, N free]); matmuls contract over partitions.

v2: big GEMMs run in fp8e4 (e4m3) with MatmulPerfMode.DoubleRow - two fp8
weights packed per PE cell and two moving rows streamed per cycle, ~2.3x the
f32r FLOP rate measured on HW.  Contraction tiles are processed in pairs
(lhsT [128,2,M], rhs [128,2,Nc<=256]).  QK^T keeps f32r (contraction D=64
gives no DR pairing; K=64 streams at ~2cyc/row in any dtype).

Quantization scales (folded into host-side weight prep, LN gammas and
copy-out dequant constants): activations x16, weights x64.  LN betas are
folded into the following GEMM's bias host-side (bias' = W @ beta + bias).

Softmax: no max-subtraction (scores are O(1) by construction); denominator
comes from a ones-row augmented onto v (M=65 <= 128 fits DR stationary);
exp() output is written to fp8 directly by the ACT engine.
"""

import os
import sys

for _p in ("/opt/trn_rl_repo",):
    if _p not in sys.path:
        sys.path.insert(0, _p)
os.environ.setdefault("MYCRO_LOCAL_CACHE", "1")

import numpy as np  # noqa: E402
import ml_dtypes  # noqa: E402

import concourse.bacc as bacc  # noqa: E402
import concourse.tile as tile  # noqa: E402
from concourse import mybir  # noqa: E402
from concourse.bass_utils import run_bass_kernel_spmd  # noqa: E402

f32 = mybir.dt.float32
f32r = mybir.dt.float32r
fp8 = mybir.dt.float8e4
u8 = mybir.dt.uint8
AF = mybir.ActivationFunctionType
ALU = mybir.AluOpType
DR = mybir.MatmulPerfMode.DoubleRow

B, N, C = 8, 1024, 1024
H, D = 16, 64
HID = 4 * C
CT = C // 128           # 8 c-tiles
CP = CT // 2            # 4 c-tile pairs
NT = N // 128           # 8 n/k-tiles
HT = HID // 128         # 32 hid-tiles
HP = HT // 2            # 16 hid-tile pairs
SCALE = D ** -0.5
EPS = 1e-5
MASK_NEG = -10000.0

S_X = 16.0              # LN output scale (folded into gamma)
S_W = 64.0              # weight scale
S_V = 16.0              # v scale
S_Y = 16.0              # attn-out scale
S_H = 1.0               # gelu-out scale (h is O(1), no post-scale op needed)

# MLP GEMM dtypes: "fp8" (DoubleRow, ~2.3x PE rate, ~1.7e-2 rel err) or
# "bf16" (f32r-rate, ~2.4e-3).  Attention path is always fp8 (~2.3e-3).
TRUNC = int(os.environ.get("TRUNC", "0"))
MLP1_DT = os.environ.get("MLP1_DT", "fp8")
MLP2_DT = os.environ.get("MLP2_DT", "bf16")
LNEXP = int(os.environ.get("LNEXP", "1"))
FP8_MLP1 = MLP1_DT == "fp8"
FP8_MLP2 = MLP2_DT == "fp8"
bf16 = mybir.dt.bfloat16

NCORES = 8


def _ln_stats(nc, tc, srcf, onesP, epsc, work, ps, tag):
    """Returns rstdB [128,N] (broadcast) and meanB for LN over partition dim
    of the 8 srcf(ct) tiles; squares on Pool, rstd = exp(-0.5*ln(var+eps))
    on ACT (stays in the natural_log_exp table set shared with attention)."""
    ps_sum = ps.tile([128, N], f32, tag="lnsum", name=f"ps_sum_{tag}")
    ps_sq = ps.tile([128, N], f32, tag="lnsq", name=f"ps_sq_{tag}")
    sq_tiles = []
    for ct in range(CT):
        sq = work.tile([128, N], f32r, tag="lnsqt", bufs=CT, name=f"sq{tag}{ct}")
        nc.gpsimd.tensor_mul(sq, srcf(ct), srcf(ct))
        sq_tiles.append(sq)
    for ch in range(2):
        cs = slice(ch * 512, (ch + 1) * 512)
        for ct in range(CT):
            nc.tensor.matmul(ps_sum[:, cs], lhsT=onesP, rhs=srcf(ct)[:, cs],
                             start=(ct == 0), stop=(ct == CT - 1))
        for ct in range(CT):
            nc.tensor.matmul(ps_sq[:, cs], lhsT=onesP, rhs=sq_tiles[ct][:, cs],
                             start=(ct == 0), stop=(ct == CT - 1))
    meanB = work.tile([128, N], f32, tag="meanB", name=f"meanB_{tag}")
    nc.vector.tensor_scalar_mul(meanB, ps_sum, 1.0 / C)
    msq = work.tile([128, N], f32, tag="msq", name=f"msq_{tag}")
    nc.vector.tensor_mul(msq, meanB, meanB)
    varB = work.tile([128, N], f32, tag="varB", name=f"varB_{tag}")
    nc.vector.scalar_tensor_tensor(varB, in0=ps_sq, scalar=1.0 / C, in1=msq,
                                   op0=ALU.mult, op1=ALU.subtract)
    if LNEXP:
        lnv = work.tile([128, N], f32, tag="lnv", name=f"lnv_{tag}")
        nc.scalar.activation(out=lnv, in_=varB, func=AF.Ln, bias=epsc,
                             scale=1.0)
        rstdB = work.tile([128, N], f32, tag="rstdB", name=f"rstdB_{tag}")
        nc.scalar.activation(out=rstdB, in_=lnv, func=AF.Exp, scale=-0.5)
    else:
        rstdB = work.tile([128, N], f32, tag="rstdB", name=f"rstdB_{tag}")
        nc.scalar.activation(out=rstdB, in_=varB, func=AF.Rsqrt, bias=epsc,
                             scale=1.0)
    return meanB, rstdB


def build_program(repeat=1):
    nc = bacc.Bacc("TRN2", target_bir_lowering=False, debug=False)

    xT = nc.dram_tensor("xT", [C, N], f32, kind="ExternalInput").ap()
    maskv = nc.dram_tensor("maskv", [N], f32, kind="ExternalInput").ap()
    g1s = nc.dram_tensor("g1s", [C], f32, kind="ExternalInput").ap()
    g2s = nc.dram_tensor("g2s", [C], f32, kind="ExternalInput").ap()
    qkvb = nc.dram_tensor("qkvb", [2 * C], f32, kind="ExternalInput").ap()
    vb = nc.dram_tensor("vb", [1, C], f32, kind="ExternalInput").ap()
    projb = nc.dram_tensor("projb", [C], f32, kind="ExternalInput").ap()
    mlp1b = nc.dram_tensor("mlp1b", [HID], f32, kind="ExternalInput").ap()
    mlp2b = nc.dram_tensor("mlp2b", [C], f32, kind="ExternalInput").ap()
    # fp8 weights as uint8 bytes (bitcast on chip)
    wqk_d = nc.dram_tensor("wqk_d", [16, 128, 1024], u8,
                           kind="ExternalInput").ap()
    wv_d = nc.dram_tensor("wv_d", [128, 8192], u8, kind="ExternalInput").ap()
    wp_d = nc.dram_tensor("wp_d", [8, 128, 1024], u8,
                          kind="ExternalInput").ap()
    if FP8_MLP1:
        w1_d = nc.dram_tensor("w1_d", [32, 128, 1024], u8,
                              kind="ExternalInput").ap()
    else:
        w1_d = nc.dram_tensor("w1_d", [32, 128, 8, 128], bf16,
                              kind="ExternalInput").ap()
    if FP8_MLP2:
        w2_d = nc.dram_tensor("w2_d", [8, 128, 4096], u8,
                              kind="ExternalInput").ap()
    else:
        w2_d = nc.dram_tensor("w2_d", [8, 128, HT, 128], bf16,
                              kind="ExternalInput").ap()
    onesd = nc.dram_tensor("onesd", [1, 128], f32, kind="ExternalInput").ap()
    outT = nc.dram_tensor("outT", [C, N], f32, kind="ExternalOutput").ap()

    DQ_QK = SCALE / (S_X * S_W) ** 2        # exp scale (dequant q*k)
    DQ_V = S_V / (S_X * S_W)                # v copy-out scale
    DQ_Y = S_Y / S_V                        # yt normalize scale
    DQ_P = 1.0 / (S_Y * S_W)                # proj dequant
    DQ_M1 = 1.0 / (S_X * S_W) if FP8_MLP1 else 1.0   # gelu pre-scale
    DQ_M2 = (1.0 / (S_H * S_W)) if FP8_MLP2 else 1.0  # mlp2 dequant

    def rg(ap_, half):
        return ap_[half * 512:(half + 1) * 512, :].rearrange(
            "(a p) f -> p a f", p=128)

    with tile.TileContext(nc) as tc:
        const_cm = tc.tile_pool(name="const", bufs=1)
        const = const_cm.__enter__()

        def vec_tiles(src_ap, n_t, name):
            t = const.tile([128, n_t], f32, name=name)
            nc.sync.dma_start(out=t, in_=src_ap.rearrange("(t p) -> p t", p=128))
            return t

        g1c = vec_tiles(g1s, CT, "g1c")
        g2c = vec_tiles(g2s, CT, "g2c")
        qkvbc = vec_tiles(qkvb, 16, "qkvbc")
        # v bias varies along the free dim of the nt-major v psum ->
        # broadcast to all 128 partitions (values already in S_V units)
        vbB = const.tile([128, 1, C], f32, name="vbB")
        nc.sync.dma_start(out=vbB, in_=vb.partition_broadcast(128))
        projbc = vec_tiles(projb, CT, "projbc")
        mlp1bc = vec_tiles(mlp1b, HT, "mlp1bc")
        mlp2bc = vec_tiles(mlp2b, CT, "mlp2bc")
        maskc = vec_tiles(maskv, NT, "maskc")
        m01 = const.tile([128, 1], f32, name="m01")
        nc.vector.memset(m01[0:D, :], 1.0)
        nc.vector.memset(m01[D:128, :], 0.0)
        m10 = const.tile([128, 1], f32, name="m10")
        nc.vector.memset(m10[0:D, :], 0.0)
        nc.vector.memset(m10[D:128, :], 1.0)
        onesP = const.tile([128, 128], f32r, name="onesP")
        nc.sync.dma_start(out=onesP,
                          in_=onesd.partition_broadcast(128).bitcast(f32r))
        epsc = const.tile([128, 1], f32, name="epsc")
        nc.vector.memset(epsc, EPS)

        for _rep in range(repeat):
            # x resident for the whole attention phase: f32r view + f32 bits
            px_cm = tc.tile_pool(name="p_x", bufs=1, side="left")
            px = px_cm.__enter__()
            xtsB = []
            for g in range(2):
                t = px.tile([128, 4, N], f32r, tag="xts", bufs=2,
                            name=f"xts{g}")
                nc.scalar.dma_start(out=t, in_=rg(xT, g).bitcast(f32r))
                xtsB.append(t)

            def xslice(ct):
                return xtsB[ct // 4][:, ct % 4, :]

            def xslice_f32(ct):
                return xtsB[ct // 4].bitcast(f32)[:, ct % 4, :]

            # ==================== LN1 (x -> xn fp8) ==================
            pln1_cm = tc.tile_pool(name="p_ln1", bufs=1, side="left")
            pln1 = pln1_cm.__enter__()
            xn_all = pln1.tile([128, CT, N], fp8, name="xn_all")
            lnw_cm = tc.tile_pool(name="lnw", bufs=1)
            lnw = lnw_cm.__enter__()
            lnp_cm = tc.tile_pool(name="lnp", bufs=1, space="PSUM")
            lnp = lnp_cm.__enter__()
            meanB, rstdB = _ln_stats(nc, tc, xslice, onesP, epsc, lnw,
                                     lnp, "ln1")
            for ct in range(CT):
                d = lnw.tile([128, N], f32, tag="lnd", bufs=3,
                             name=f"lnd1_{ct}")
                nc.gpsimd.tensor_sub(d, xslice_f32(ct), meanB)
                nc.vector.scalar_tensor_tensor(
                    xn_all[:, ct, :], in0=d, scalar=g1c[:, ct:ct + 1],
                    in1=rstdB, op0=ALU.mult, op1=ALU.mult)
            lnp_cm.__exit__(None, None, None)
            lnw_cm.__exit__(None, None, None)

            # ======================== QKV ============================
            pattn_cm = tc.tile_pool(name="p_attn", bufs=1, side="right")
            pattn = pattn_cm.__enter__()
            qkt = [pattn.tile([128, N], f32r, tag="qkt", bufs=8,
                              name=f"qkt{i}") for i in range(8)]
            # k for head 2j/2j+1 zero-padded to full 128 partitions so the
            # QK moving side streams 128-partition rows (1cyc/row, not the
            # 2cyc/row K=64 penalty): kz[2j] = [k_2j; 0], kz[2j+1] = [0; k_..]
            kz = [pattn.tile([128, N], f32r, tag="kz", bufs=16,
                             name=f"kz{i}") for i in range(16)]
            # vkt: per kt-pair [128, 2, 16*(64+1)] fp8
            vkt = [pattn.tile([128, 2, H * (D + 1)], fp8, tag="vkt", bufs=4,
                              name=f"vkt{kp}") for kp in range(4)]
            for kp in range(4):
                vcol = vkt[kp].rearrange("p two (h u) -> p two h u", u=D + 1)
                nc.gpsimd.memset(vcol[:, :, :, D:D + 1], 1.0)

            wq_cm = tc.tile_pool(name="wq_pool", bufs=1)
            wqp = wq_cm.__enter__()
            qps_cm = tc.tile_pool(name="qkv_ps", bufs=1, space="PSUM")
            qps = qps_cm.__enter__()

            # weights: v first ([128, 8192] fp8 = 8KB/p), then q/k in 4-tile
            # groups [128, 4, 1024]
            wv_s = wqp.tile([128, 4, 2, 1024], fp8, name="wv_s")
            nc.sync.dma_start(out=wv_s,
                             in_=wv_d.bitcast(fp8).rearrange(
                                 "p (cp two e) -> p cp two e", cp=4, two=2))
            wqk_s = {}
            for grp in range(4):  # q: grp 0,1 ; k: grp 2,3
                t = wqp.tile([128, 4, 1024], fp8, tag="wqk", bufs=4,
                             name=f"wqk{grp}")
                nc.scalar.dma_start(
                    out=t, in_=wqk_d[4 * grp:4 * grp + 4].bitcast(fp8)
                    .rearrange("a p f -> p a f"))
                wqk_s[grp] = t

            def wqk_lhsT(j, cp):
                # f-tile j (0-7 q, 8-15 k), c-pair cp -> [128, 2, 128]
                t = wqk_s[j // 4]
                return t[:, j % 4, cp * 256:(cp + 1) * 256].rearrange(
                    "p (two m) -> p two m", two=2)

            def xn_pair(cp, fs):
                return xn_all[:, 2 * cp:2 * cp + 2, fs]

            # v in nt-major: out [128 npos, 1024 vdims]
            for nt in range(NT):
                ps = qps.tile([128, N], f32, tag="qkvps", bufs=3,
                              name=f"vps{nt}")
                for oc in range(4):
                    ocs = slice(oc * 256, (oc + 1) * 256)
                    for cp in range(CP):
                        nc.tensor.matmul(
                            ps[:, ocs],
                            lhsT=xn_pair(cp, slice(nt * 128, nt * 128 + 128)),
                            rhs=wv_s[:, cp, :, ocs],
                            start=(cp == 0), stop=(cp == CP - 1),
                            perf_mode=DR)
                dst = vkt[nt // 2][:, nt % 2, :].rearrange(
                    "p (h u) -> p h u", u=D + 1)[:, :, 0:D]
                nc.vector.scalar_tensor_tensor(
                    dst, in0=ps.rearrange("p (h u) -> p h u", u=D),
                    scalar=DQ_V,
                    in1=vbB[:, 0, :].rearrange("p (h u) -> p h u", u=D),
                    op0=ALU.mult, op1=ALU.add)
            # q/k interleaved per head-pair j: q f-tile j then k f-tile j.
            # Engine split: q bias-add + kz even-half copy on ACT (Identity
            # with per-partition bias); kz odd-half on DVE (masked add);
            # kz even-half zero region memset on Pool.
            for j in range(8):
                # f32r Memset is invalid ISA; zero via x*0 from resident x
                nc.vector.tensor_scalar_mul(kz[2 * j][D:128, :],
                                            xtsB[0][D:128, 0, :], 0.0)
                for qk in range(2):
                    ft = 8 * qk + j
                    ps = qps.tile([128, N], f32, tag="qkvps", bufs=3,
                                  name=f"qkps{ft}")
                    for oc in range(4):
                        ocs = slice(oc * 256, (oc + 1) * 256)
                        for cp in range(CP):
                            nc.tensor.matmul(
                                ps[:, ocs], lhsT=wqk_lhsT(ft, cp),
                                rhs=xn_pair(cp, ocs),
                                start=(cp == 0), stop=(cp == CP - 1),
                                perf_mode=DR)
                    if qk == 0:
                        nc.scalar.activation(out=qkt[j], in_=ps,
                                             func=AF.Identity,
                                             bias=qkvbc[:, ft:ft + 1],
                                             scale=1.0)
                    else:
                        nc.scalar.activation(
                            out=kz[2 * j][0:D, :],
                            in_=ps[0:D, :], func=AF.Identity,
                            bias=qkvbc[0:D, ft:ft + 1], scale=1.0)
                        nc.vector.tensor_scalar(
                            kz[2 * j + 1], in0=ps,
                            scalar1=qkvbc[:, ft:ft + 1],
                            scalar2=m10, op0=ALU.add, op1=ALU.mult)

            qps_cm.__exit__(None, None, None)
            wq_cm.__exit__(None, None, None)
            pln1_cm.__exit__(None, None, None)

            if TRUNC == 1:
                for g in range(2):
                    nc.scalar.dma_start(
                        out=rg(outT, g),
                        in_=xtsB[g].bitcast(f32))
                pattn_cm.__exit__(None, None, None)
                px_cm.__exit__(None, None, None)
                continue

            # ====================== attention ========================
            pyt_cm = tc.tile_pool(name="p_yt", bufs=1, side="left")
            pyt = pyt_cm.__enter__()
            yt_all = pyt.tile([128, CT, N], fp8, name="yt_all")
            asb_cm = tc.tile_pool(name="attn_sb", bufs=1)
            asb = asb_cm.__enter__()
            aps_cm = tc.tile_pool(name="attn_ps", bufs=1, space="PSUM")
            aps = aps_cm.__enter__()

            if TRUNC == 35:
                nc.vector.memset(yt_all.bitcast(u8), 56)  # ~0.006 in e4m3
            for h in range(H if TRUNC != 35 else 0):
                j, half = h // 2, h % 2
                qk_q = qkt[j]
                qk_k = kz[h]
                ya = aps.tile([D + 1, N], f32, tag="ya", bufs=2,
                              name=f"ya{h}")
                eas = []
                for kp in range(4):
                    ea = asb.tile([128, 2, N], fp8, tag="ea", bufs=6,
                                  name=f"ea{h}_{kp}")
                    for par in range(2):
                        kt = 2 * kp + par
                        sa = aps.tile([128, N], f32, tag="sa", bufs=2,
                                      name=f"sa{h}_{kt}")
                        ks = slice(kt * 128, (kt + 1) * 128)
                        for ch in range(2):
                            cs = slice(ch * 512, (ch + 1) * 512)
                            nc.tensor.matmul(sa[:, cs], lhsT=qk_k[:, ks],
                                             rhs=qk_q[:, cs],
                                             start=True, stop=True)
                        nc.scalar.activation(out=ea[:, par, :], in_=sa,
                                             func=AF.Exp,
                                             bias=maskc[:, kt:kt + 1],
                                             scale=DQ_QK)
                    eas.append(ea)
                # chunk-major AV so each 1KB psum zero-region sees exactly
                # one start..stop accumulation group
                for oc in range(4):
                    ocs = slice(oc * 256, (oc + 1) * 256)
                    for kp in range(4):
                        va = vkt[kp][:, :, h * 65:h * 65 + 65]
                        nc.tensor.matmul(ya[:, ocs], lhsT=va,
                                         rhs=eas[kp][:, :, ocs],
                                         start=(kp == 0), stop=(kp == 3),
                                         perf_mode=DR)
                # pack the head pair into full-128-partition tiles so the
                # yt_all write is a single full-partition op (partial-
                # partition writes into a matmul-consumed tile stall HW)
                if half == 0:
                    yu2 = asb.tile([128, N], f32, tag="yu2", bufs=2,
                                   name=f"yu2_{j}")
                if TRUNC == 25:
                    nc.vector.tensor_copy(yu2[64 * half:64 * half + 64, :],
                                          ya[0:D, :])
                    continue
                rf = asb.tile([1, N], f32, tag="rf", bufs=3,
                              name=f"rf_{h}")
                nc.vector.reciprocal(rf, ya[D:D + 1, :])
                # denominator reciprocal broadcast across partitions on Pool
                # (base-64 partial broadcast is broken on HW; use full range)
                rbh = asb.tile([128, N], f32, tag="rbh", bufs=3,
                               name=f"rbh_{h}")
                nc.gpsimd.partition_broadcast(rbh, rf[0:1, :])
                # normalize folded into the ya -> yu2 pack copy
                nc.vector.scalar_tensor_tensor(
                    yu2[64 * half:64 * half + 64, :], in0=ya[0:D, :],
                    scalar=DQ_Y, in1=rbh[64 * half:64 * half + 64, :],
                    op0=ALU.mult, op1=ALU.mult)
                if half == 1:
                    nc.vector.tensor_copy(yt_all[:, j, :], yu2)

            aps_cm.__exit__(None, None, None)
            asb_cm.__exit__(None, None, None)
            pattn_cm.__exit__(None, None, None)

            if TRUNC in (2, 25):
                for g in range(2):
                    nc.scalar.dma_start(
                        out=rg(outT, g), in_=xtsB[g].bitcast(f32))
                pyt_cm.__exit__(None, None, None)
                px_cm.__exit__(None, None, None)
                continue

            # =================== proj + residual =====================
            px2_cm = tc.tile_pool(name="p_x2", bufs=1, side="right")
            px2 = px2_cm.__enter__()
            x2_all = px2.tile([128, CT, N], f32r, name="x2_all")

            wp_cm = tc.tile_pool(name="wp_pool", bufs=1)
            wpp = wp_cm.__enter__()
            pps_cm = tc.tile_pool(name="proj_ps", bufs=1, space="PSUM")
            pps = pps_cm.__enter__()

            wp_s = {}
            for grp in range(2):
                t = wpp.tile([128, 4, 1024], fp8, tag="wp", bufs=2,
                             name=f"wp{grp}")
                nc.sync.dma_start(
                    out=t, in_=wp_d[4 * grp:4 * grp + 4].bitcast(fp8)
                    .rearrange("a p f -> p a f"))
                wp_s[grp] = t

            def yt_pair(cp, fs):
                return yt_all[:, 2 * cp:2 * cp + 2, fs]

            xpb = []  # x + bproj staging (Pool)
            for o in range(CT):
                t = wpp.tile([128, N], f32, tag="xpb", bufs=4,
                             name=f"xpb{o}")
                nc.gpsimd.tensor_scalar_add(t, xslice_f32(o),
                                            projbc[:, o:o + 1])
                xpb.append(t)
            for o in range(CT):
                ps = pps.tile([128, N], f32, tag="projps", bufs=3,
                              name=f"pps{o}")
                wt = wp_s[o // 4]
                for oc in range(4):
                    ocs = slice(oc * 256, (oc + 1) * 256)
                    for cp in range(CP):
                        lhsT = wt[:, o % 4, cp * 256:(cp + 1) * 256].rearrange(
                            "p (two m) -> p two m", two=2)
                        nc.tensor.matmul(ps[:, ocs], lhsT=lhsT,
                                         rhs=yt_pair(cp, ocs),
                                         start=(cp == 0), stop=(cp == CP - 1),
                                         perf_mode=DR)
                nc.vector.scalar_tensor_tensor(
                    x2_all[:, o, :], in0=ps, scalar=DQ_P, in1=xpb[o],
                    op0=ALU.mult, op1=ALU.add)
            pps_cm.__exit__(None, None, None)
            wp_cm.__exit__(None, None, None)
            pyt_cm.__exit__(None, None, None)
            px_cm.__exit__(None, None, None)

            if TRUNC == 3:
                for g in range(2):
                    nc.scalar.dma_start(
                        out=rg(outT, g),
                        in_=x2_all.bitcast(f32)[:, 4 * g:4 * g + 4, :])
                px2_cm.__exit__(None, None, None)
                continue

            def x2slice(ct):
                return x2_all[:, ct, :]

            def x2slice_f32(ct):
                return x2_all.bitcast(f32)[:, ct, :]

            # =================== LN2 (x2 -> x2n) =====================
            px2n_cm = tc.tile_pool(name="p_x2n", bufs=1, side="left")
            px2n = px2n_cm.__enter__()
            x2n_all = px2n.tile([128, CT, N], fp8 if FP8_MLP1 else bf16,
                                name="x2n_all")
            ln2w_cm = tc.tile_pool(name="ln2w", bufs=1)
            ln2w = ln2w_cm.__enter__()
            ln2p_cm = tc.tile_pool(name="ln2p", bufs=1, space="PSUM")
            ln2p = ln2p_cm.__enter__()
            meanB2, rstdB2 = _ln_stats(nc, tc, x2slice, onesP, epsc,
                                       ln2w, ln2p, "ln2")
            for ct in range(CT):
                d = ln2w.tile([128, N], f32, tag="lnd", bufs=3,
                              name=f"lnd2_{ct}")
                nc.gpsimd.tensor_sub(d, x2slice_f32(ct), meanB2)
                nc.vector.scalar_tensor_tensor(
                    x2n_all[:, ct, :], in0=d, scalar=g2c[:, ct:ct + 1],
                    in1=rstdB2, op0=ALU.mult, op1=ALU.mult)
            ln2p_cm.__exit__(None, None, None)
            ln2w_cm.__exit__(None, None, None)

            # ========================= MLP ===========================
            pmlp_cm = tc.tile_pool(name="p_mlp", bufs=1, side="right")
            pmlp = pmlp_cm.__enter__()
            h_all = pmlp.tile([128, HT, N], fp8 if FP8_MLP2 else bf16,
                              name="h_all")
            mw_cm = tc.tile_pool(name="mw_pool", bufs=1)
            mwp = mw_cm.__enter__()
            mps_cm = tc.tile_pool(name="mlp_ps", bufs=1, space="PSUM")
            mps = mps_cm.__enter__()

            def x2n_pair(cp, fs):
                return x2n_all[:, 2 * cp:2 * cp + 2, fs]

            # ---- fc1 + gelu ----
            for grp in range(8):     # 4 f-tiles per group
                if FP8_MLP1:
                    w1tiles = None
                    wt = mwp.tile([128, 4, 1024], fp8, tag="w1", bufs=3,
                                  name=f"w1_{grp}")
                    nc.sync.dma_start(
                        out=wt, in_=w1_d[4 * grp:4 * grp + 4].bitcast(fp8)
                        .rearrange("a p f -> p a f"))
                else:
                    wt = mwp.tile([128, 4, CT, 128], bf16, tag="w1", bufs=3,
                                  name=f"w1_{grp}")
                    nc.sync.dma_start(
                        out=wt, in_=w1_d[4 * grp:4 * grp + 4].rearrange(
                            "a p c m -> p a c m"))
                for fl in range(4):
                    f = 4 * grp + fl
                    ps = mps.tile([128, N], f32, tag="mlp1ps", bufs=2,
                                  name=f"m1ps{f}")
                    if FP8_MLP1:
                        for oc in range(4):
                            ocs = slice(oc * 256, (oc + 1) * 256)
                            for cp in range(CP):
                                lhsT = wt[:, fl,
                                          cp * 256:(cp + 1) * 256].rearrange(
                                    "p (two m) -> p two m", two=2)
                                nc.tensor.matmul(
                                    ps[:, ocs], lhsT=lhsT,
                                    rhs=x2n_pair(cp, ocs),
                                    start=(cp == 0), stop=(cp == CP - 1),
                                    perf_mode=DR)
                    else:
                        for ct in range(CT):
                            for ch in range(2):
                                cs = slice(ch * 512, (ch + 1) * 512)
                                nc.tensor.matmul(
                                    ps[:, cs],
                                    lhsT=wt[:, fl, ct, :],
                                    rhs=x2n_all[:, ct, cs],
                                    start=(ct == 0), stop=(ct == CT - 1))
                    nc.scalar.activation(out=h_all[:, f, :], in_=ps,
                                         func=AF.Gelu,
                                         bias=mlp1bc[:, f:f + 1],
                                         scale=DQ_M1)

            # ---- fc2 + residual + out ----
            x2pb = []
            for o in range(CT):
                t = mwp.tile([128, N], f32, tag="x2pb", bufs=4,
                             name=f"x2pb{o}")
                nc.gpsimd.tensor_scalar_add(t, x2slice_f32(o),
                                            mlp2bc[:, o:o + 1])
                x2pb.append(t)

            def h_pair(fp_, fs):
                return h_all[:, 2 * fp_:2 * fp_ + 2, fs]

            osb_cm = tc.tile_pool(name="out_sb", bufs=1)
            osb = osb_cm.__enter__()
            ots = [osb.tile([128, 4, N], f32, tag="ot", bufs=2,
                            name=f"ot{g}") for g in range(2)]
            for o in range(CT):
                if FP8_MLP2:
                    wt = mwp.tile([128, 16, 2, 128], fp8, tag="w2", bufs=2,
                                  name=f"w2_{o}")
                    nc.scalar.dma_start(
                        out=wt, in_=w2_d[o].bitcast(fp8).rearrange(
                            "p (fp two m) -> p fp two m", fp=16, two=2))
                    ps = mps.tile([128, N], f32, tag="mlp2ps", bufs=2,
                                  name=f"m2ps{o}")
                    for oc in range(4):
                        ocs = slice(oc * 256, (oc + 1) * 256)
                        for fp_ in range(HP):
                            nc.tensor.matmul(
                                ps[:, ocs], lhsT=wt[:, fp_],
                                rhs=h_pair(fp_, ocs),
                                start=(fp_ == 0), stop=(fp_ == HP - 1),
                                perf_mode=DR)
                else:
                    wt = mwp.tile([128, HT, 128], bf16, tag="w2", bufs=2,
                                  name=f"w2_{o}")
                    nc.scalar.dma_start(out=wt, in_=w2_d[o])
                    ps = mps.tile([128, N], f32, tag="mlp2ps", bufs=2,
                                  name=f"m2ps{o}")
                    for ft in range(HT):
                        for ch in range(2):
                            cs = slice(ch * 512, (ch + 1) * 512)
                            nc.tensor.matmul(
                                ps[:, cs], lhsT=wt[:, ft, :],
                                rhs=h_all[:, ft, cs],
                                start=(ft == 0), stop=(ft == HT - 1))
                nc.vector.scalar_tensor_tensor(
                    ots[o // 4][:, o % 4, :], in0=ps, scalar=DQ_M2,
                    in1=x2pb[o], op0=ALU.mult, op1=ALU.add)
                if o % 4 == 3:
                    nc.scalar.dma_start(out=rg(outT, o // 4),
                                        in_=ots[o // 4])
            osb_cm.__exit__(None, None, None)
            mps_cm.__exit__(None, None, None)
            mw_cm.__exit__(None, None, None)
            px2n_cm.__exit__(None, None, None)
            pmlp_cm.__exit__(None, None, None)
            px2_cm.__exit__(None, None, None)

        const_cm.__exit__(None, None, None)

    nc.compile()
    return nc


_NC_CACHE = {}


def _get_program():
    if "nc" not in _NC_CACHE:
        _NC_CACHE["nc"] = build_program()
    return _NC_CACHE["nc"]


def _q8(a, scale):
    """numpy f32 -> e4m3 bytes (uint8 view) with scale."""
    return np.asarray(np.asarray(a, np.float64) * scale,
                      dtype=ml_dtypes.float8_e4m3).view(np.uint8)


def _dr_tiles(WT_rows, scale):
    """[128*nf, K] weight rows -> [nf, 128, K/256, 2, 128] DR lhsT bytes,
    flattened to [nf, 128, K*nf_bytes...] layout [nf, 128, (cp two m)]."""
    M, K = WT_rows.shape
    nf = M // 128
    a = WT_rows.reshape(nf, 128, K // 256, 2, 128)   # [nf, m, cp, i, p]
    a = a.transpose(0, 4, 2, 3, 1)                   # [nf, p, cp, i, m]
    return np.ascontiguousarray(_q8(a, scale).reshape(nf, 128, K))


def build_in_maps(ins):
    """Host-side input prep shared by kernel() and time_hw.py."""
    x = np.asarray(ins["x"], dtype=np.float32)
    length = np.asarray(ins["length"])
    g1 = np.asarray(ins["g1"], np.float32)
    b1 = np.asarray(ins["b1"], np.float32)
    g2 = np.asarray(ins["g2"], np.float32)
    b2 = np.asarray(ins["b2"], np.float32)
    bproj = np.asarray(ins["bproj"], np.float32)
    bb1 = np.asarray(ins["bb1"], np.float32)
    bb2 = np.asarray(ins["bb2"], np.float32)
    Wqkv = np.asarray(ins["Wqkv"], np.float32)
    Wproj = np.asarray(ins["Wproj"], np.float32)
    W1 = np.asarray(ins["W1"], np.float32)
    W2 = np.asarray(ins["W2"], np.float32)

    # LN beta folded into following GEMM bias (scaled to PSUM units)
    qkv_bias = Wqkv @ b1                       # [3C]
    qkvb = (qkv_bias[0:2 * C]) * (S_X * S_W)   # q,k rows (PSUM units)
    vb_host = np.zeros((C,), np.float32) + qkv_bias[2 * C:3 * C]  # v rows
    mlp1b = bb1 + W1 @ b2                      # true units (ACT bias)

    wqk = _dr_tiles(Wqkv[0:2 * C], S_W)        # [16, 128, 1024]
    wv_rows = Wqkv[2 * C:3 * C]                # [C, C] (e', c)
    a = wv_rows.T.reshape(4, 2, 128, C).transpose(2, 0, 1, 3)  # [p,cp,i,e']
    wv = np.ascontiguousarray(_q8(a, S_W).reshape(128, 8192))
    wp = _dr_tiles(Wproj, S_W)                 # [8, 128, 1024]
    if FP8_MLP1:
        w1 = _dr_tiles(W1, S_W)                # [32, 128, 1024]
    else:
        w1 = np.ascontiguousarray(
            W1.reshape(32, 128, 8, 128).transpose(0, 3, 2, 1)
            .astype(ml_dtypes.bfloat16))       # [32, 128, CT, 128]
    if FP8_MLP2:
        a = W2.reshape(8, 128, 16, 2, 128).transpose(0, 4, 2, 3, 1)
        w2 = np.ascontiguousarray(_q8(a, S_W).reshape(8, 128, 4096))
    else:
        w2 = np.ascontiguousarray(
            W2.reshape(8, 128, HT, 128).transpose(0, 3, 2, 1)
            .astype(ml_dtypes.bfloat16))       # [8, 128, HT, 128]

    xT = np.ascontiguousarray(x.transpose(0, 2, 1))  # [B, C, N]
    mask = (np.arange(N)[None, :] >= length[:, None]).astype(
        np.float32) * MASK_NEG  # [B, N]

    shared = {
        "g1s": g1 * S_X, "g2s": g2 * (S_X if FP8_MLP1 else 1.0),
        "qkvb": qkvb, "vb": (vb_host * S_V).reshape(1, C),
        "projb": bproj, "mlp1b": mlp1b, "mlp2b": bb2,
        "wqk_d": wqk, "wv_d": wv, "wp_d": wp, "w1_d": w1, "w2_d": w2,
        "onesd": np.ones((1, 128), np.float32),
    }
    return [dict(shared, xT=xT[b], maskv=np.ascontiguousarray(mask[b]))
            for b in range(B)]


def kernel(x, length, g1, b1, Wqkv, Wproj, bproj, g2, b2, W1, bb1, W2, bb2):
    in_maps = build_in_maps(dict(
        x=x, length=length, g1=g1, b1=b1, Wqkv=Wqkv, Wproj=Wproj,
        bproj=bproj, g2=g2, b2=b2, W1=W1, bb1=bb1, W2=W2, bb2=bb2))

    nc = _get_program()
    res = run_bass_kernel_spmd(nc, in_maps, core_ids=list(range(NCORES)))
    out = np.stack([res.results[b]["outT"] for b in range(B)], axis=0)
    return np.ascontiguousarray(out.transpose(0, 2, 1))

